# revision 12
# baseline (speedup 1.0000x reference)
"""Trainium2 Bass kernel for nn_Attention additive-attention module.

Reference computation (B=64, S=2048, H=512):
    u1    = tgt @ Wq_w.T + Wq_b                       # (B, H)
    u2    = einsum("oh,bsh->bos", Wref_w, src) + Wref_b[:, None]   # (B, H, S)
    logit = einsum("h,bhs->bs", V, tanh(u1[:, :, None] + u2))      # (B, S)
    probs = softmax(logit, axis=-1)
    d'    = einsum("bhs,bs->bh", u2, probs)
    return (d_prime, probs, logit)

Strategy: data-parallel over batch, 8 batches per NeuronCore on 8 cores,
bf16 on the TensorEngine with f32 accumulation.

Per (batch, s-chunk of 128): a PSUM tile [s=128, o=512] accumulates a
K=128 selector matmul (e_b row-selector x u1 rows, injecting u1_aug =
tgt@Wq^T + Wq_b + Wref_b broadcast over s; K=128 keeps the PE MM pipeline
bubble-free) plus four K=128 matmuls (src^T chunk stationary, Wref^T
moving), giving (u1+u2)^T. ScalarE tanh reads PSUM directly -> t [s,o]
bf16. The V-contraction: GpSimd multiplies by a V row broadcast across
partitions, VectorE free-dim-reduces, accumulating logits transposed
[s%128, s//128]. Softmax skips max-subtraction (|logit| <= sum|V_h| ~ 20,
exp is safe in f32). Pass 2 uses the UNNORMALIZED exp values (already
transposed for the M=1 stationary): c = sum_s exp[s]*src[s,:], scaled by
1/sum on evacuation; d' = Wref@c + Wref_b (sum(probs)=1 folds the conv
bias). Pass 2 and the output transposes are software-pipelined one batch
behind the main loop so the PE never waits on the softmax chain.

Host-side prep (sharding strategy): src is sharded on batch and supplied
in both layouts ([H,S] transposed for the main matmul, [S,H] natural for
the probs-weighted reduction) as bf16; small params are replicated,
pre-transposed and pre-cast; identity/selector constants ship as inputs.
"""

import sys

if "/opt/trn_rl_repo" not in sys.path:
    sys.path.insert(0, "/opt/trn_rl_repo")

import numpy as np
import ml_dtypes

import concourse.bass as bass
import concourse.mybir as mybir
import concourse.tile as tile
from concourse import bacc
from concourse.bass_utils import run_bass_kernel_spmd

BF16 = ml_dtypes.bfloat16

B, S, H = 64, 2048, 512
NCORES = 8
BSH = B // NCORES          # 8 batches per core
HC = H // 128              # 4 h chunks
SC = S // 128              # 16 s chunks of 128

_cache = {}


def _build_program():
    nc = bacc.Bacc(
        "TRN2",
        target_bir_lowering=False,
        debug=False,
        enable_asserts=True,
        num_devices=NCORES,
    )
    f32 = mybir.dt.float32
    bf16 = mybir.dt.bfloat16
    Tanh = mybir.ActivationFunctionType.Tanh
    Exp = mybir.ActivationFunctionType.Exp
    from concourse.bass_isa import ReduceOp

    # Per-core I/O
    idm = nc.dram_tensor("idm", (128, 128), bf16, kind="ExternalInput").ap()
    e8 = nc.dram_tensor("e8", (128, BSH, 128), bf16, kind="ExternalInput").ap()
    tgtT = nc.dram_tensor("tgtT", (H, BSH), bf16, kind="ExternalInput").ap()
    wqT = nc.dram_tensor("wqT", (H, H), bf16, kind="ExternalInput").ap()
    bias1 = nc.dram_tensor("bias1", (1, H), bf16, kind="ExternalInput").ap()
    wrb = nc.dram_tensor("wrb", (1, H), bf16, kind="ExternalInput").ap()
    vbc = nc.dram_tensor("vbc", (128, H), bf16, kind="ExternalInput").ap()
    wrT = nc.dram_tensor("wrT", (H, H), bf16, kind="ExternalInput").ap()
    srcT = nc.dram_tensor("srcT", (BSH, H, S), bf16, kind="ExternalInput").ap()
    srcN = nc.dram_tensor("srcN", (BSH, S, H), bf16, kind="ExternalInput").ap()

    d_out = nc.dram_tensor("d_out", (BSH, H), f32, kind="ExternalOutput").ap()
    p_out = nc.dram_tensor("p_out", (BSH, S), f32, kind="ExternalOutput").ap()
    l_out = nc.dram_tensor("l_out", (BSH, S), f32, kind="ExternalOutput").ap()

    with tile.TileContext(nc) as tc:
        with (
            tc.tile_pool(name="const", bufs=1) as cpool,
            tc.tile_pool(name="sT", bufs=2) as sTpool,
            tc.tile_pool(name="sN", bufs=3) as sNpool,
            tc.tile_pool(name="t", bufs=4) as tpool,
            tc.tile_pool(name="lg", bufs=2) as lpool,
            tc.tile_pool(name="sm", bufs=3) as smpool,
            tc.tile_pool(name="pb", bufs=2) as pbpool,
            tc.tile_pool(name="pu2", bufs=4, space="PSUM") as pu2pool,
            tc.tile_pool(name="pmisc", bufs=2, space="PSUM") as pmiscpool,
        ):
            # ---- constants / weights (small first so compute starts early) ----
            id_bf = cpool.tile([128, 128], bf16, tag="idb")
            nc.sync.dma_start(out=id_bf[:], in_=idm)
            tgtT_sb = cpool.tile([128, HC, BSH], bf16, tag="tgt")
            nc.sync.dma_start(out=tgtT_sb[:], in_=tgtT.rearrange("(c p) b -> p c b", p=128))
            wqT_sb = cpool.tile([128, HC, H], bf16, tag="wq")
            wq3 = wqT.rearrange("(c p) o -> p c o", p=128)
            for hc in range(HC):
                nc.sync.dma_start(out=wqT_sb[:, hc, :], in_=wq3[:, hc, :])
            e8_sb = cpool.tile([128, BSH, 128], bf16, tag="e8")
            nc.sync.dma_start(out=e8_sb[:], in_=e8)
            bias1_sb = cpool.tile([1, H], bf16, tag="b1")
            nc.sync.dma_start(out=bias1_sb[:], in_=bias1)
            wrb_sb = cpool.tile([1, H], bf16, tag="wb")
            nc.sync.dma_start(out=wrb_sb[:], in_=wrb)
            vbc_sb = cpool.tile([128, H], bf16, tag="vb")
            nc.sync.dma_start(out=vbc_sb[:], in_=vbc)
            wrT_sb = cpool.tile([128, HC, H], bf16, tag="wr")
            nc.sync.dma_start(out=wrT_sb[:], in_=wrT.rearrange("(c p) o -> p c o", p=128))

            ones18 = cpool.tile([1, BSH], bf16, tag="on8")
            nc.vector.memset(ones18[:], 1.0)
            zb = cpool.tile([128, 1], f32, tag="zb")
            nc.vector.memset(zb[:], 0.0)

            # PE warmup: dense dummy matmuls during the DMA prologue so the
            # HAM clock gate is released before real work arrives. Uses a
            # memset tile so warmup has no DMA dependency.
            wk_sb = cpool.tile([128, 128], bf16, tag="wk")
            nc.vector.memset(wk_sb[:], 0.0)
            ps_w = pmiscpool.tile([128, 128], f32, tag="pm")
            for _ in range(24):
                nc.tensor.matmul(
                    ps_w[:], lhsT=wk_sb[:], rhs=wk_sb[:], start=True, stop=True
                )

            # cT8[:, hc, b] = c_b^T chunks (c_b = probs-weighted src sum)
            cT8_sb = cpool.tile([128, HC, BSH], bf16, tag="cT8")

            # ---- u1_aug = tgt @ Wq^T + Wq_b + Wref_b : rows 0-7 of u1_88 ----
            ps_u1 = pmiscpool.tile([BSH, H], f32, tag="pm")
            for hc in range(HC):
                nc.tensor.matmul(
                    ps_u1[:],
                    lhsT=tgtT_sb[:, hc, :],
                    rhs=wqT_sb[:, hc, :],
                    start=(hc == 0),
                    stop=False,
                )
            nc.tensor.matmul(
                ps_u1[:], lhsT=ones18[:], rhs=bias1_sb[:], start=False, stop=True
            )
            u1_88 = cpool.tile([128, H], bf16, tag="u188")
            nc.vector.memset(u1_88[:], 0.0)
            nc.vector.tensor_copy(u1_88[0:BSH, :], ps_u1[:])

            # software-pipeline state: batch b's softmax-dependent work runs
            # during batch b+1's matmul stream
            pending = []  # (b, expt_bf, sN_sb, rbc, probs_t, logit_t)

            def emit_deferred(b, expt_bf, sN_sb, rbc, probs_t, logit_t):
                # pass 2 on unnormalized exp values; scale on evacuation
                ps_c = pmiscpool.tile([1, H], f32, tag="pm")
                for sc in range(SC):
                    nc.tensor.matmul(
                        ps_c[:],
                        lhsT=expt_bf[:, sc:sc + 1],
                        rhs=sN_sb[:, sc, :],
                        start=(sc == 0),
                        stop=(sc == SC - 1),
                    )
                c_sb = smpool.tile([1, H], bf16, tag="cb")
                nc.vector.tensor_scalar_mul(c_sb[:], ps_c[:], rbc[0:1, 0:1])
                # transpose the c row into cT8 columns via K=1 matmuls
                for hc in range(HC):
                    ps_ct = pmiscpool.tile([128, 1], f32, tag="pm")
                    nc.tensor.matmul(
                        ps_ct[:],
                        lhsT=c_sb[:, hc * 128:(hc + 1) * 128],
                        rhs=ones18[:, 0:1],
                        start=True, stop=True,
                    )
                    nc.vector.tensor_copy(cT8_sb[:, hc, b:b + 1], ps_ct[:])

                # outputs: bf16 transpose [128,16] -> [16,128], contiguous DMA
                probs_bf = pbpool.tile([128, SC], bf16, tag="ptb")
                nc.vector.tensor_copy(probs_bf[:], probs_t[:])
                ps_po = pmiscpool.tile([16, 128], bf16, tag="pm")
                nc.tensor.transpose(ps_po[:], probs_bf[:], id_bf[:])
                po_sb = smpool.tile([16, 128], f32, tag="po")
                nc.vector.tensor_copy(po_sb[:], ps_po[:])
                nc.scalar.dma_start(out=p_out[b:b + 1, :], in_=po_sb[:])

                logit_bf = smpool.tile([128, SC], bf16, tag="lgb")
                nc.vector.tensor_copy(logit_bf[:], logit_t[:])
                ps_lo = pmiscpool.tile([16, 128], bf16, tag="pm")
                nc.tensor.transpose(ps_lo[:], logit_bf[:], id_bf[:])
                lo_sb = smpool.tile([16, 128], f32, tag="lo")
                nc.vector.tensor_copy(lo_sb[:], ps_lo[:])
                nc.scalar.dma_start(out=l_out[b:b + 1, :], in_=lo_sb[:])

            for b in range(BSH):
                # src^T chunks, one tile per h-chunk so the first matmul only
                # waits on a quarter of the transfer
                sT_tiles = []
                for hc in range(HC):
                    sTk = sTpool.tile([128, S], bf16, tag=f"sT{hc}")
                    nc.sync.dma_start(
                        out=sTk[:], in_=srcT[b][hc * 128:(hc + 1) * 128, :]
                    )
                    sT_tiles.append(sTk)

                # logits, transposed: [s%128, s//128]
                logit_t = lpool.tile([128, SC], f32, tag="lgt")

                for sc in range(SC):
                    # deferred work of the previous batch, emitted mid-stream
                    if sc == SC - 4 and pending:
                        emit_deferred(*pending.pop())

                    ps_u2 = pu2pool.tile([128, H], f32, tag="psu2")
                    nc.tensor.matmul(
                        ps_u2[:], lhsT=e8_sb[:, b, :], rhs=u1_88[:],
                        start=True, stop=False,
                    )
                    for hc in range(HC):
                        nc.tensor.matmul(
                            ps_u2[:],
                            lhsT=sT_tiles[hc][:, sc * 128:(sc + 1) * 128],
                            rhs=wrT_sb[:, hc, :],
                            start=False,
                            stop=(hc == HC - 1),
                        )
                    t_sb = tpool.tile([128, H], bf16, tag="t")
                    nc.scalar.activation(t_sb[:], ps_u2[:], Tanh, bias=zb[:])
                    # logit chunk = sum_o V[o] * t[s, o] in one fused DVE op
                    tv_sb = tpool.tile([128, H], bf16, tag="tv")
                    nc.vector.affine_mul_reduce(
                        out=tv_sb[:], accum_out=logit_t[:, sc:sc + 1],
                        in0=t_sb[:], in1=vbc_sb[:], scale=1.0, bias=0.0,
                    )

                # natural-layout src for pass 2 (needed one batch later)
                sN_sb = sNpool.tile([128, SC, H], bf16, tag="sN")
                nc.sync.dma_start(
                    out=sN_sb[:], in_=srcN[b].rearrange("(c p) h -> p c h", p=128)
                )

                # ---- softmax over s, on transposed [128, 16] layout ----
                # |logit| <= sum|V_h| (~20): exp without max-subtraction is
                # safe in f32.
                expt = smpool.tile([128, SC], f32, tag="ex")
                nc.scalar.activation(expt[:], logit_t[:], Exp, bias=zb[:])
                expt_bf = pbpool.tile([128, SC], bf16, tag="exb")
                nc.vector.tensor_copy(expt_bf[:], expt[:])
                sumpart = smpool.tile([128, 1], f32, tag="sp")
                nc.vector.tensor_reduce(
                    sumpart[:], expt[:], axis=mybir.AxisListType.X,
                    op=mybir.AluOpType.add,
                )
                stot = smpool.tile([128, 1], f32, tag="st")
                nc.gpsimd.partition_all_reduce(
                    stot[:], sumpart[:], 128, ReduceOp.add
                )
                rbc = smpool.tile([128, 1], f32, tag="rb")
                nc.vector.reciprocal(rbc[:], stot[:])
                probs_t = pbpool.tile([128, SC], f32, tag="pt")
                nc.vector.tensor_scalar_mul(probs_t[:], expt[:], rbc[:])

                pending.append((b, expt_bf, sN_sb, rbc, probs_t, logit_t))

            # ---- tail: last batch deferred work, then d' = Wref@c + Wref_b ----
            emit_deferred(*pending.pop())

            ps_d = pmiscpool.tile([BSH, H], f32, tag="pm")
            for hc in range(HC):
                nc.tensor.matmul(
                    ps_d[:],
                    lhsT=cT8_sb[:, hc, :],
                    rhs=wrT_sb[:, hc, :],
                    start=(hc == 0),
                    stop=False,
                )
            nc.tensor.matmul(
                ps_d[:], lhsT=ones18[:], rhs=wrb_sb[:], start=False, stop=True
            )
            d_sb = smpool.tile([BSH, H], f32, tag="dsb")
            nc.vector.tensor_copy(d_sb[:], ps_d[:])
            nc.scalar.dma_start(out=d_out[:], in_=d_sb[:])

    nc.compile()
    return nc


def _get_program():
    if "nc" not in _cache:
        _cache["nc"] = _build_program()
    return _cache["nc"]


def _make_in_maps(src, tgt, Wq_w, Wq_b, Wref_w, Wref_b, V):
    wqT = np.ascontiguousarray(Wq_w.T).astype(BF16)
    wrT = np.ascontiguousarray(Wref_w.T).astype(BF16)
    bias1 = (Wq_b + Wref_b).reshape(1, H).astype(BF16)
    wrb = Wref_b.reshape(1, H).astype(BF16)
    vbc = np.ascontiguousarray(
        np.broadcast_to(V.reshape(1, H), (128, H))
    ).astype(BF16)
    idm = np.eye(128, dtype=BF16)
    e8 = np.zeros((128, BSH, 128), dtype=BF16)
    for b in range(BSH):
        e8[b, b, :] = 1.0

    in_maps = []
    for c in range(NCORES):
        bs = slice(c * BSH, (c + 1) * BSH)
        src_c = src[bs]
        in_maps.append({
            "idm": idm,
            "e8": e8,
            "srcT": np.ascontiguousarray(src_c.transpose(0, 2, 1)).astype(BF16),
            "srcN": np.ascontiguousarray(src_c).astype(BF16),
            "tgtT": np.ascontiguousarray(tgt[bs].T).astype(BF16),
            "wqT": wqT,
            "wrT": wrT,
            "bias1": bias1,
            "wrb": wrb,
            "vbc": vbc,
        })
    return in_maps


def run_sharded(inputs, trace=False, **kwargs):
    """Run the SPMD kernel; returns (outputs_tuple, BassKernelResults)."""
    inputs = {k: np.asarray(v) for k, v in inputs.items()}
    nc = _get_program()
    in_maps = _make_in_maps(**inputs)
    res = run_bass_kernel_spmd(
        nc, in_maps, core_ids=list(range(NCORES)), trace=trace, **kwargs
    )
    d = np.concatenate([r["d_out"] for r in res.results], axis=0)
    p = np.concatenate([r["p_out"] for r in res.results], axis=0)
    l = np.concatenate([r["l_out"] for r in res.results], axis=0)
    return (d, p, l), res


def kernel(**inputs):
    outs, _ = run_sharded(inputs)
    return outs


# revision 13
# speedup vs baseline: 1.0286x; 1.0286x over previous
"""Trainium2 Bass kernel for nn_Attention additive-attention module.

Reference computation (B=64, S=2048, H=512):
    u1    = tgt @ Wq_w.T + Wq_b                       # (B, H)
    u2    = einsum("oh,bsh->bos", Wref_w, src) + Wref_b[:, None]   # (B, H, S)
    logit = einsum("h,bhs->bs", V, tanh(u1[:, :, None] + u2))      # (B, S)
    probs = softmax(logit, axis=-1)
    d'    = einsum("bhs,bs->bh", u2, probs)
    return (d_prime, probs, logit)

Strategy: data-parallel over batch, 8 batches per NeuronCore on 8 cores,
bf16 on the TensorEngine with f32 accumulation.

Per (batch, s-chunk of 128): a PSUM tile [s=128, o=512] accumulates a
K=128 selector matmul (e_b row-selector x u1 rows, injecting u1_aug =
tgt@Wq^T + Wq_b + Wref_b broadcast over s; K=128 keeps the PE MM pipeline
bubble-free) plus four K=128 matmuls (src^T chunk stationary, Wref^T
moving), giving (u1+u2)^T. ScalarE tanh reads PSUM directly -> t [s,o]
bf16. The V-contraction: GpSimd multiplies by a V row broadcast across
partitions, VectorE free-dim-reduces, accumulating logits transposed
[s%128, s//128]. Softmax skips max-subtraction (|logit| <= sum|V_h| ~ 20,
exp is safe in f32). Pass 2 uses the UNNORMALIZED exp values (already
transposed for the M=1 stationary): c = sum_s exp[s]*src[s,:], scaled by
1/sum on evacuation; d' = Wref@c + Wref_b (sum(probs)=1 folds the conv
bias). Pass 2 and the output transposes are software-pipelined one batch
behind the main loop so the PE never waits on the softmax chain.

Host-side prep (sharding strategy): src is sharded on batch and supplied
in both layouts ([H,S] transposed for the main matmul, [S,H] natural for
the probs-weighted reduction) as bf16; small params are replicated,
pre-transposed and pre-cast; identity/selector constants ship as inputs.
"""

import sys

if "/opt/trn_rl_repo" not in sys.path:
    sys.path.insert(0, "/opt/trn_rl_repo")

import numpy as np
import ml_dtypes

import concourse.bass as bass
import concourse.mybir as mybir
import concourse.tile as tile
from concourse import bacc
from concourse.bass_utils import run_bass_kernel_spmd

BF16 = ml_dtypes.bfloat16

B, S, H = 64, 2048, 512
NCORES = 8
BSH = B // NCORES          # 8 batches per core
HC = H // 128              # 4 h chunks
SC = S // 128              # 16 s chunks of 128

_cache = {}


def _build_program():
    nc = bacc.Bacc(
        "TRN2",
        target_bir_lowering=False,
        debug=False,
        enable_asserts=True,
        num_devices=NCORES,
    )
    f32 = mybir.dt.float32
    bf16 = mybir.dt.bfloat16
    Tanh = mybir.ActivationFunctionType.Tanh
    Exp = mybir.ActivationFunctionType.Exp
    from concourse.bass_isa import ReduceOp

    # Per-core I/O
    idm = nc.dram_tensor("idm", (128, 128), bf16, kind="ExternalInput").ap()
    e8 = nc.dram_tensor("e8", (128, BSH, 128), bf16, kind="ExternalInput").ap()
    tgtT = nc.dram_tensor("tgtT", (H, BSH), bf16, kind="ExternalInput").ap()
    wqT = nc.dram_tensor("wqT", (H, H), bf16, kind="ExternalInput").ap()
    bias1 = nc.dram_tensor("bias1", (1, H), bf16, kind="ExternalInput").ap()
    wrb = nc.dram_tensor("wrb", (1, H), bf16, kind="ExternalInput").ap()
    vbc = nc.dram_tensor("vbc", (128, H), bf16, kind="ExternalInput").ap()
    wrT = nc.dram_tensor("wrT", (H, H), bf16, kind="ExternalInput").ap()
    srcT = nc.dram_tensor("srcT", (BSH, H, S), bf16, kind="ExternalInput").ap()
    srcN = nc.dram_tensor("srcN", (BSH, S, H), bf16, kind="ExternalInput").ap()

    d_out = nc.dram_tensor("d_out", (BSH, H), f32, kind="ExternalOutput").ap()
    p_out = nc.dram_tensor("p_out", (BSH, S), f32, kind="ExternalOutput").ap()
    l_out = nc.dram_tensor("l_out", (BSH, S), f32, kind="ExternalOutput").ap()

    with tile.TileContext(nc) as tc:
        with (
            tc.tile_pool(name="const", bufs=1) as cpool,
            tc.tile_pool(name="sT", bufs=2) as sTpool,
            tc.tile_pool(name="sN", bufs=3) as sNpool,
            tc.tile_pool(name="t", bufs=4) as tpool,
            tc.tile_pool(name="lg", bufs=2) as lpool,
            tc.tile_pool(name="sm", bufs=3) as smpool,
            tc.tile_pool(name="pb", bufs=2) as pbpool,
            tc.tile_pool(name="pu2", bufs=4, space="PSUM") as pu2pool,
            tc.tile_pool(name="pmisc", bufs=2, space="PSUM") as pmiscpool,
        ):
            # ---- constants / weights (small first so compute starts early) ----
            id_bf = cpool.tile([128, 128], bf16, tag="idb")
            nc.sync.dma_start(out=id_bf[:], in_=idm)
            e8_sb = cpool.tile([128, BSH, 128], bf16, tag="e8")
            nc.sync.dma_start(out=e8_sb[:], in_=e8)
            tgtT_sb = cpool.tile([128, HC, BSH], bf16, tag="tgt")
            nc.sync.dma_start(out=tgtT_sb[:], in_=tgtT.rearrange("(c p) b -> p c b", p=128))
            wqT_sb = cpool.tile([128, HC, H], bf16, tag="wq")
            nc.sync.dma_start(out=wqT_sb[:], in_=wqT.rearrange("(c p) o -> p c o", p=128))
            bias1_sb = cpool.tile([1, H], bf16, tag="b1")
            nc.sync.dma_start(out=bias1_sb[:], in_=bias1)
            wrb_sb = cpool.tile([1, H], bf16, tag="wb")
            nc.sync.dma_start(out=wrb_sb[:], in_=wrb)
            vbc_sb = cpool.tile([128, H], bf16, tag="vb")
            nc.sync.dma_start(out=vbc_sb[:], in_=vbc)
            wrT_sb = cpool.tile([128, HC, H], bf16, tag="wr")
            nc.sync.dma_start(out=wrT_sb[:], in_=wrT.rearrange("(c p) o -> p c o", p=128))

            ones18 = cpool.tile([1, BSH], bf16, tag="on8")
            nc.vector.memset(ones18[:], 1.0)
            zb = cpool.tile([128, 1], f32, tag="zb")
            nc.vector.memset(zb[:], 0.0)

            # PE warmup: dense dummy matmuls during the DMA prologue so the
            # HAM clock gate is released before real work arrives. Uses a
            # memset tile so warmup has no DMA dependency.
            wk_sb = cpool.tile([128, 128], bf16, tag="wk")
            nc.vector.memset(wk_sb[:], 0.0)
            ps_w = pmiscpool.tile([128, 128], f32, tag="pm")
            for _ in range(24):
                nc.tensor.matmul(
                    ps_w[:], lhsT=wk_sb[:], rhs=wk_sb[:], start=True, stop=True
                )

            # cT8[:, hc, b] = c_b^T chunks (c_b = probs-weighted src sum)
            cT8_sb = cpool.tile([128, HC, BSH], bf16, tag="cT8")

            # ---- u1_aug = tgt @ Wq^T + Wq_b + Wref_b : rows 0-7 of u1_88 ----
            ps_u1 = pmiscpool.tile([BSH, H], f32, tag="pm")
            for hc in range(HC):
                nc.tensor.matmul(
                    ps_u1[:],
                    lhsT=tgtT_sb[:, hc, :],
                    rhs=wqT_sb[:, hc, :],
                    start=(hc == 0),
                    stop=False,
                )
            nc.tensor.matmul(
                ps_u1[:], lhsT=ones18[:], rhs=bias1_sb[:], start=False, stop=True
            )
            u1_88 = cpool.tile([128, H], bf16, tag="u188")
            nc.vector.memset(u1_88[:], 0.0)
            nc.vector.tensor_copy(u1_88[0:BSH, :], ps_u1[:])

            # software-pipeline state: batch b's softmax-dependent work runs
            # during batch b+1's matmul stream
            pending = []  # (b, expt_bf, sN_sb, rbc, probs_t, logit_t)

            def emit_deferred(b, expt_bf, sN_sb, rbc, probs_t, logit_t):
                # pass 2 on unnormalized exp values; scale on evacuation
                ps_c = pmiscpool.tile([1, H], f32, tag="pm")
                for sc in range(SC):
                    nc.tensor.matmul(
                        ps_c[:],
                        lhsT=expt_bf[:, sc:sc + 1],
                        rhs=sN_sb[:, sc, :],
                        start=(sc == 0),
                        stop=(sc == SC - 1),
                    )
                c_sb = smpool.tile([1, H], bf16, tag="cb")
                nc.vector.tensor_scalar_mul(c_sb[:], ps_c[:], rbc[0:1, 0:1])
                # transpose the c row into cT8 columns via K=1 matmuls
                for hc in range(HC):
                    ps_ct = pmiscpool.tile([128, 1], f32, tag="pm")
                    nc.tensor.matmul(
                        ps_ct[:],
                        lhsT=c_sb[:, hc * 128:(hc + 1) * 128],
                        rhs=ones18[:, 0:1],
                        start=True, stop=True,
                    )
                    nc.vector.tensor_copy(cT8_sb[:, hc, b:b + 1], ps_ct[:])

                # outputs: bf16 transpose [128,16] -> [16,128], contiguous DMA
                probs_bf = pbpool.tile([128, SC], bf16, tag="ptb")
                nc.vector.tensor_copy(probs_bf[:], probs_t[:])
                ps_po = pmiscpool.tile([16, 128], bf16, tag="pm")
                nc.tensor.transpose(ps_po[:], probs_bf[:], id_bf[:])
                po_sb = smpool.tile([16, 128], f32, tag="po")
                nc.vector.tensor_copy(po_sb[:], ps_po[:])
                nc.scalar.dma_start(out=p_out[b:b + 1, :], in_=po_sb[:])

                logit_bf = smpool.tile([128, SC], bf16, tag="lgb")
                nc.vector.tensor_copy(logit_bf[:], logit_t[:])
                ps_lo = pmiscpool.tile([16, 128], bf16, tag="pm")
                nc.tensor.transpose(ps_lo[:], logit_bf[:], id_bf[:])
                lo_sb = smpool.tile([16, 128], f32, tag="lo")
                nc.vector.tensor_copy(lo_sb[:], ps_lo[:])
                nc.scalar.dma_start(out=l_out[b:b + 1, :], in_=lo_sb[:])

            for b in range(BSH):
                # src^T chunks, one tile per h-chunk so the first matmul only
                # waits on a quarter of the transfer
                sT_tiles = []
                for hc in range(HC):
                    sTk = sTpool.tile([128, S], bf16, tag=f"sT{hc}")
                    nc.sync.dma_start(
                        out=sTk[:], in_=srcT[b][hc * 128:(hc + 1) * 128, :]
                    )
                    sT_tiles.append(sTk)

                # logits, transposed: [s%128, s//128]
                logit_t = lpool.tile([128, SC], f32, tag="lgt")

                for sc in range(SC):
                    # deferred work of the previous batch, emitted mid-stream
                    if sc == SC - 4 and pending:
                        emit_deferred(*pending.pop())

                    ps_u2 = pu2pool.tile([128, H], f32, tag="psu2")
                    nc.tensor.matmul(
                        ps_u2[:], lhsT=e8_sb[:, b, :], rhs=u1_88[:],
                        start=True, stop=False,
                    )
                    for hc in range(HC):
                        nc.tensor.matmul(
                            ps_u2[:],
                            lhsT=sT_tiles[hc][:, sc * 128:(sc + 1) * 128],
                            rhs=wrT_sb[:, hc, :],
                            start=False,
                            stop=(hc == HC - 1),
                        )
                    t_sb = tpool.tile([128, H], bf16, tag="t")
                    nc.scalar.activation(t_sb[:], ps_u2[:], Tanh, bias=zb[:])
                    # logit chunk = sum_o V[o] * t[s, o] in one fused DVE op
                    tv_sb = tpool.tile([128, H], bf16, tag="tv")
                    nc.vector.affine_mul_reduce(
                        out=tv_sb[:], accum_out=logit_t[:, sc:sc + 1],
                        in0=t_sb[:], in1=vbc_sb[:], scale=1.0, bias=0.0,
                    )

                # natural-layout src for pass 2 (needed one batch later)
                sN_sb = sNpool.tile([128, SC, H], bf16, tag="sN")
                nc.sync.dma_start(
                    out=sN_sb[:], in_=srcN[b].rearrange("(c p) h -> p c h", p=128)
                )

                # ---- softmax over s, on transposed [128, 16] layout ----
                # |logit| <= sum|V_h| (~20): exp without max-subtraction is
                # safe in f32.
                expt = smpool.tile([128, SC], f32, tag="ex")
                nc.scalar.activation(expt[:], logit_t[:], Exp, bias=zb[:])
                expt_bf = pbpool.tile([128, SC], bf16, tag="exb")
                nc.vector.tensor_copy(expt_bf[:], expt[:])
                sumpart = smpool.tile([128, 1], f32, tag="sp")
                nc.vector.tensor_reduce(
                    sumpart[:], expt[:], axis=mybir.AxisListType.X,
                    op=mybir.AluOpType.add,
                )
                stot = smpool.tile([128, 1], f32, tag="st")
                nc.gpsimd.partition_all_reduce(
                    stot[:], sumpart[:], 128, ReduceOp.add
                )
                rbc = smpool.tile([128, 1], f32, tag="rb")
                nc.vector.reciprocal(rbc[:], stot[:])
                probs_t = pbpool.tile([128, SC], f32, tag="pt")
                nc.vector.tensor_scalar_mul(probs_t[:], expt[:], rbc[:])

                pending.append((b, expt_bf, sN_sb, rbc, probs_t, logit_t))

            # ---- tail: last batch deferred work, then d' = Wref@c + Wref_b ----
            emit_deferred(*pending.pop())

            ps_d = pmiscpool.tile([BSH, H], f32, tag="pm")
            for hc in range(HC):
                nc.tensor.matmul(
                    ps_d[:],
                    lhsT=cT8_sb[:, hc, :],
                    rhs=wrT_sb[:, hc, :],
                    start=(hc == 0),
                    stop=False,
                )
            nc.tensor.matmul(
                ps_d[:], lhsT=ones18[:], rhs=wrb_sb[:], start=False, stop=True
            )
            d_sb = smpool.tile([BSH, H], f32, tag="dsb")
            nc.vector.tensor_copy(d_sb[:], ps_d[:])
            nc.scalar.dma_start(out=d_out[:], in_=d_sb[:])

    nc.compile()
    return nc


def _get_program():
    if "nc" not in _cache:
        _cache["nc"] = _build_program()
    return _cache["nc"]


def _make_in_maps(src, tgt, Wq_w, Wq_b, Wref_w, Wref_b, V):
    wqT = np.ascontiguousarray(Wq_w.T).astype(BF16)
    wrT = np.ascontiguousarray(Wref_w.T).astype(BF16)
    bias1 = (Wq_b + Wref_b).reshape(1, H).astype(BF16)
    wrb = Wref_b.reshape(1, H).astype(BF16)
    vbc = np.ascontiguousarray(
        np.broadcast_to(V.reshape(1, H), (128, H))
    ).astype(BF16)
    idm = np.eye(128, dtype=BF16)
    e8 = np.zeros((128, BSH, 128), dtype=BF16)
    for b in range(BSH):
        e8[b, b, :] = 1.0

    in_maps = []
    for c in range(NCORES):
        bs = slice(c * BSH, (c + 1) * BSH)
        src_c = src[bs]
        in_maps.append({
            "idm": idm,
            "e8": e8,
            "srcT": np.ascontiguousarray(src_c.transpose(0, 2, 1)).astype(BF16),
            "srcN": np.ascontiguousarray(src_c).astype(BF16),
            "tgtT": np.ascontiguousarray(tgt[bs].T).astype(BF16),
            "wqT": wqT,
            "wrT": wrT,
            "bias1": bias1,
            "wrb": wrb,
            "vbc": vbc,
        })
    return in_maps


def run_sharded(inputs, trace=False, **kwargs):
    """Run the SPMD kernel; returns (outputs_tuple, BassKernelResults)."""
    inputs = {k: np.asarray(v) for k, v in inputs.items()}
    nc = _get_program()
    in_maps = _make_in_maps(**inputs)
    res = run_bass_kernel_spmd(
        nc, in_maps, core_ids=list(range(NCORES)), trace=trace, **kwargs
    )
    d = np.concatenate([r["d_out"] for r in res.results], axis=0)
    p = np.concatenate([r["p_out"] for r in res.results], axis=0)
    l = np.concatenate([r["l_out"] for r in res.results], axis=0)
    return (d, p, l), res


def kernel(**inputs):
    outs, _ = run_sharded(inputs)
    return outs
